# revision 5
# baseline (speedup 1.0000x reference)
"""Depthwise-separable conv (3x3 depthwise rank-1 + 1x1 pointwise) on 8
Trainium2 NeuronCores.

Sharding: data-parallel over batch — 2 images per core.

Per-core algorithm, per 32-row slab (C=128 channels on partitions):
  1. DMA the slab of x (with 1-row halo) into SBUF.
  2. Column conv (3-tap along H, per-channel scalars): ACT does the center
     tap (activation Copy with per-partition scale), DVE accumulates the
     two shifted taps in-place via scalar_tensor_tensor. Output y1 is
     rounded to float32r (required for the fast fp32r matmul path).
  3. Row conv + pointwise are folded into the PE: out[o,h,w] =
     sum_j (pw[o,c]*row[c,j]) y1[c,h,w+j-1] — 3 accumulated float32r
     matmuls per 512-element PSUM chunk, with w-shifts expressed as
     offset access patterns (edge columns get zero-pad semantics for free
     by narrowing the APs).
  4. PSUM chunks are evacuated to SBUF by ACT/DVE (alternating), then
     DMA'd to the output.
"""
import sys

sys.path.insert(0, "/opt/trn_rl_repo")

from contextlib import ExitStack

import numpy as np

import concourse.bass as bass
import concourse.tile as tile
from concourse import bacc, mybir
from concourse.bass_utils import run_bass_kernel_spmd

F32 = mybir.dt.float32
F32R = mybir.dt.float32r

B, C, H, W = 16, 128, 128, 128
OUT = 256
N_CORES = 8
B_LOC = B // N_CORES          # images per core
SLAB = 32                     # rows per slab
N_SLABS = H // SLAB
CHUNK = 512                   # psum chunk = 4 rows of W=128
N_CHUNK = SLAB * W // CHUNK   # 8 chunks per slab

LAST_EXEC_NS = None

_CACHED_NC = None


def _build():
    nc = bacc.Bacc(trn_type="TRN2", target_bir_lowering=False, debug=False)
    xin = nc.dram_tensor("xin", [B_LOC, C, H, W], F32, kind="ExternalInput").ap()
    wfold = nc.dram_tensor("wfold", [3, C, OUT], F32, kind="ExternalInput").ap()
    colk = nc.dram_tensor("colk", [C, 3], F32, kind="ExternalInput").ap()
    out = nc.dram_tensor("out", [B_LOC, OUT, H, W], F32, kind="ExternalOutput").ap()

    with tile.TileContext(nc) as tc, ExitStack() as ctx:
        wpool = ctx.enter_context(tc.tile_pool(name="weights", bufs=1))
        xpool = ctx.enter_context(tc.tile_pool(name="x", bufs=3))
        ypool = ctx.enter_context(tc.tile_pool(name="y1", bufs=3))
        opool = ctx.enter_context(tc.tile_pool(name="out", bufs=4))
        pspool = ctx.enter_context(tc.tile_pool(name="ps", bufs=8, space="PSUM"))

        # --- weights: DMA fp32, round to f32r on DVE (fp32r matmul operands
        # must be produced by a rounding compute op, not a DMA)
        w_f32 = wpool.tile([C, 3 * OUT], F32, tag="w32")
        for j in range(3):
            nc.sync.dma_start(w_f32[:, j * OUT:(j + 1) * OUT], wfold[j])
        w_r = wpool.tile([C, 3 * OUT], F32R, tag="wr")
        nc.vector.tensor_copy(w_r[:], w_f32[:])
        ck = wpool.tile([C, 3], F32, tag="ck")
        nc.sync.dma_start(ck[:], colk[:])

        def wj(j, oc):  # lhsT [C=128, O=128] for tap j, out-channel half oc
            return w_r[:, j * OUT + oc * 128: j * OUT + oc * 128 + 128]

        for b in range(B_LOC):
            for s in range(N_SLABS):
                h0 = s * SLAB
                # --- x slab with halo: rows h0-1 .. h0+SLAB (SLAB+2 rows)
                x_t = xpool.tile([C, (SLAB + 2) * W], F32, tag="xs")
                if s == 0:
                    nc.gpsimd.memset(x_t[:, 0:W], 0.0)
                    nc.sync.dma_start(x_t[:, W:], xin[b, :, 0:SLAB + 1, :])
                elif s == N_SLABS - 1:
                    nc.gpsimd.memset(x_t[:, (SLAB + 1) * W:], 0.0)
                    nc.sync.dma_start(x_t[:, 0:(SLAB + 1) * W],
                                      xin[b, :, h0 - 1:H, :])
                else:
                    nc.sync.dma_start(x_t[:], xin[b, :, h0 - 1:h0 + SLAB + 1, :])

                # --- column conv -> y1 (f32r), stored with row stride W+1:
                # y1[h][w] at offset 1 + h*(W+1) + w. The inter-row pad
                # column (offset h*(W+1)) is zeroed, which gives the row
                # conv zero-pad edge semantics with full-width, fp32r-legal
                # access patterns: tap j of chunk rows r0..r0+3 is simply
                # yp[:, r0:r0+4, j:j+W].
                WP = W + 1
                n_el = SLAB * W
                y1 = ypool.tile([C, (SLAB + 1) * WP + 2], F32R, tag="y1")
                yp = y1[:, 0:(SLAB + 1) * WP].rearrange("c (h w) -> c h w", w=WP)
                # tap-j shifted row view: ypj(j)[:, r, w] = y1 flat[j + r*WP + w]
                ypj = [y1[:, j:j + (SLAB + 1) * WP]
                       .rearrange("c (h w) -> c h w", w=WP) for j in range(3)]
                nc.gpsimd.memset(yp[:, :, 0:1].bitcast(F32), 0.0)
                yd = yp[:, 0:SLAB, 1:WP]          # data view [C, 32, 128]
                x3 = x_t[:].rearrange("c (h w) -> c h w", w=W)
                nc.scalar.activation(yd, x3[:, 1:SLAB + 1, :],
                                     mybir.ActivationFunctionType.Copy,
                                     scale=ck[:, 1:2])
                nc.vector.scalar_tensor_tensor(
                    yd, x3[:, 0:SLAB, :], ck[:, 0:1], yd,
                    op0=mybir.AluOpType.mult, op1=mybir.AluOpType.add)
                nc.vector.scalar_tensor_tensor(
                    yd, x3[:, 2:SLAB + 2, :], ck[:, 2:3], yd,
                    op0=mybir.AluOpType.mult, op1=mybir.AluOpType.add)

                # --- row conv + pointwise folded into PE (f32r matmuls)
                RPC = CHUNK // W                  # rows per psum chunk
                for oc in range(2):
                    o_t = opool.tile([C, n_el], F32, tag="ot")
                    for q in range(N_CHUNK):
                        ps = pspool.tile([128, CHUNK], F32, tag="ps")
                        r0 = q * RPC
                        for jx, j in enumerate((0, 1, 2)):
                            nc.tensor.matmul(
                                ps[:], wj(j, oc),
                                ypj[j][:, r0:r0 + RPC, 0:W],
                                start=(jx == 0), stop=(jx == 2))
                        dst = o_t[:, q * CHUNK:(q + 1) * CHUNK]
                        if q % 2 == 0:
                            nc.scalar.copy(dst, ps[:])
                        else:
                            nc.vector.tensor_copy(dst, ps[:])
                    nc.sync.dma_start(out[b, oc * 128:(oc + 1) * 128,
                                          h0:h0 + SLAB, :], o_t[:])
    nc.compile()
    return nc


def kernel(x, col_kernel, row_kernel, pw_weight, trace=False):
    global LAST_EXEC_NS, _CACHED_NC
    x = np.ascontiguousarray(np.asarray(x, dtype=np.float32))
    colk3 = np.asarray(col_kernel, dtype=np.float32).reshape(C, 3)
    rowk3 = np.asarray(row_kernel, dtype=np.float32).reshape(C, 3)
    pw = np.asarray(pw_weight, dtype=np.float32)

    # fold row-conv taps into the pointwise weight: Wj[c, o] = pw[o,c]*row[c,j]
    wfold = np.ascontiguousarray(
        pw.T[None, :, :] * rowk3.T[:, :, None]).astype(np.float32)  # [3, C, OUT]

    if _CACHED_NC is None:
        _CACHED_NC = _build()
    nc = _CACHED_NC

    in_maps = [
        {"xin": np.ascontiguousarray(x[i * B_LOC:(i + 1) * B_LOC]),
         "wfold": wfold, "colk": np.ascontiguousarray(colk3)}
        for i in range(N_CORES)
    ]
    res = run_bass_kernel_spmd(nc, in_maps, list(range(N_CORES)), trace=trace)
    LAST_EXEC_NS = res.exec_time_ns
    return np.concatenate([res.results[i]["out"] for i in range(N_CORES)],
                          axis=0)
